# revision 10
# baseline (speedup 1.0000x reference)
"""Trainium2 Bass kernel for the DiffsolClassifier model.

Network (per image, NCHW fp32):
    z1 = relu(conv2d(x, W1, b1, k=3, s=2, p=1))   # [8,14,14]
    z2 = relu(conv2d(z1, W2, b2, k=3, s=2, p=1))  # [16,7,7]
    t  = flatten(z2) @ Wfc.T + bfc                # [1]
    p  = clip(1 - exp(-(softplus(t) + 1e-3)), 1e-6, 1-1e-6)
       = (1-k) + k*sigmoid(t),  k = exp(-1e-3)    (exact identity;
         the clip changes the result by <= 1e-6 rel so it is dropped)

Sharding: pure data parallel, batch 65536 split 8192/core across 8 cores.

Per-core mapping (16 outer tiles x 512 images), fp16 data / fp32 PSUM:
  - The HOST pre-lays x into conv1-window order: for each outer tile,
    xw[p, oi, n] = x[img n, pixel 56*oi - 28 + p] (zero when out of
    range), p in 0..83 covering the 3 input rows each conv1 output row
    reads.  On device this is a plain contiguous DMA (14336B per
    partition) - no DMA transpose and no per-window weight variants.
  - conv1: 14 matmuls per tile, ALL sharing one stationary weight
    matrix w1 [84, 112] (112 = 8ch x 14 cols, banded taps), so the PE
    never reloads weights within the phase.  Bias+relu fused into the
    PSUM eviction (alternating ACT/DVE), output z1 rows [112, 512] fp16.
  - conv2: output rows in chunks of 2; per chunk the 3 tap matrices
    w2r1/w2r0/w2r2 [112,112] are each used for both rows back-to-back
    (grouped to reuse the loaded weights), accumulating in fp32 PSUM.
    Chunks are interleaved between conv1 matmuls so the eviction
    engines (slower per-tile than the PE) never stall PSUM recycling.
  - FC: per z2 row a [112,1] matmul accumulated into PSUM [1,512]; the
    7 matmuls + sigmoid epilogue of tile t are emitted inside tile
    t+1's conv1 stream so the z2-eviction -> fc dependency is hidden.
  - All fp16 weights ship in ONE packed [128, 455] DMA and the fp32
    biases in one [112, 3] DMA, issued on the scalar queue so they
    overlap the first x-tile DMA on the sync queue.
"""

import numpy as np

B = 65536
NCORES = 8
BS = B // NCORES  # 8192 images per core
TN = 512          # images per outer tile
NT = BS // TN     # 16 outer tiles

KDEC = float(np.exp(np.float32(-0.001)))

# conv1 window geometry: window oi covers pixels 56*oi-28 .. 56*oi+55
WIN = 84   # window rows (3 image rows x 28 cols)
NWIN = 14  # conv1 output rows

# set by test.py for profiling; harness leaves these alone
TRACE = False
LAST_EXEC_NS = None
LAST_PROFILE_JSON = None


def _build_weight_mats(W1, b1, W2, b2, Wfc):
    """Host-side restructuring of the tiny conv/fc weights into the
    banded matrices the PE matmuls consume."""
    W1 = np.asarray(W1, np.float32).reshape(8, 1, 3, 3)
    W2 = np.asarray(W2, np.float32).reshape(16, 8, 3, 3)
    Wfc = np.asarray(Wfc, np.float32).reshape(1, 784)

    # w1row[(di,j), (co,oj)] over a 3-row x 28-col input window
    w1row = np.zeros((WIN, 112), np.float32)
    for co in range(8):
        for oj in range(14):
            m = co * 14 + oj
            for di in range(3):
                for dj in range(3):
                    j = 2 * oj - 1 + dj
                    if 0 <= j < 28:
                        w1row[di * 28 + j, m] = W1[co, 0, di, dj]

    # conv2 tap matrices: W2r[di][(ci,j), (co2,oj2)]
    W2r = np.zeros((3, 112, 112), np.float32)
    for di in range(3):
        for co in range(16):
            for oj in range(7):
                m = co * 7 + oj
                for ci in range(8):
                    for dj in range(3):
                        j = 2 * oj - 1 + dj
                        if 0 <= j < 14:
                            W2r[di, ci * 14 + j, m] = W2[co, ci, di, dj]

    # fc columns per z2 row: wfc[(co2,oj2), i2]
    wfc = np.zeros((112, 7), np.float32)
    for co in range(16):
        for i2 in range(7):
            for oj in range(7):
                wfc[co * 7 + oj, i2] = Wfc[0, co * 49 + i2 * 7 + oj]

    # one packed fp16 weight image [128, 455]:
    #   cols   0:112 w1row (rows 0:84)
    #   cols 112:224 w2r0, 224:336 w2r1, 336:448 w2r2 (rows 0:112)
    #   cols 448:455 wfc (rows 0:112)
    wpack = np.zeros((128, 455), np.float16)
    wpack[0:WIN, 0:112] = w1row
    wpack[0:112, 112:224] = W2r[0]
    wpack[0:112, 224:336] = W2r[1]
    wpack[0:112, 336:448] = W2r[2]
    wpack[0:112, 448:455] = wfc

    # packed fp32 per-partition consts [112, 3]: b1col | b2col | bfc@[0,2]
    cpack = np.zeros((112, 3), np.float32)
    cpack[:, 0] = np.repeat(np.asarray(b1, np.float32), 14)
    cpack[:, 1] = np.repeat(np.asarray(b2, np.float32), 7)
    return wpack, cpack


def _window_x(x16pad):
    """[B, 840] padded fp16 -> per-core tile-major window layout
    [NCORES][NT*84, 14*512] where row (t*84+p), col (oi*512+n) holds
    x[core*8192 + t*512 + n, pixel 56*oi - 28 + p]."""
    from numpy.lib.stride_tricks import as_strided
    v = as_strided(x16pad, shape=(B, NWIN, WIN),
                   strides=(840 * 2, 56 * 2, 2))
    full = v.reshape(NCORES, NT, TN, NWIN, WIN).transpose(0, 1, 4, 3, 2)
    out = []
    for c in range(NCORES):
        out.append(np.ascontiguousarray(full[c]).reshape(NT * WIN, NWIN * TN))
    return out


def _build_nc(nt_tiles):
    import concourse.bacc as bacc
    import concourse.bass as bass
    import concourse.mybir as mybir
    import concourse.tile as tile

    f32 = mybir.dt.float32
    f16 = mybir.dt.float16
    AF = mybir.ActivationFunctionType
    OP = mybir.AluOpType
    bs = nt_tiles * TN

    nc = bacc.Bacc(None)
    x_d = nc.declare_dram_parameter("xw", [nt_tiles * WIN, NWIN * TN], f16,
                                    isOutput=False)
    w_d = nc.declare_dram_parameter("wpack", [128, 455], f16, isOutput=False)
    c_d = nc.declare_dram_parameter("cpack", [112, 3], f32, isOutput=False)
    bfc_d = nc.declare_dram_parameter("bfc", [1, 1], f32, isOutput=False)
    y_d = nc.declare_dram_parameter("y", [bs], f32, isOutput=True)

    with tile.TileContext(nc) as tc:
        with (
            tc.tile_pool(name="const", bufs=1) as const,
            tc.tile_pool(name="head_pool", bufs=1) as head_pool,
            tc.tile_pool(name="xt_pool", bufs=6) as xt_pool,
            tc.tile_pool(name="z1_pool", bufs=16) as z1_pool,
            tc.tile_pool(name="z2_pool", bufs=16) as z2_pool,
            tc.tile_pool(name="y_pool", bufs=1) as y_pool,
            tc.tile_pool(name="c1_psum", bufs=4, space="PSUM") as c1_pool,
            tc.tile_pool(name="c2_psum", bufs=3, space="PSUM") as c2_pool,
            tc.tile_pool(name="fc_psum", bufs=1, space="PSUM") as fc_pool,
        ):
            wsb = const.tile([128, 455], f16, name="wsb")
            csb = const.tile([112, 3], f32, name="csb")
            bfc = const.tile([1, 1], f32, name="bfc")
            # weights on the scalar queue: overlaps the first x DMA (sync)
            nc.scalar.dma_start(out=wsb[:], in_=w_d[:])
            nc.scalar.dma_start(out=csb[:], in_=c_d[:])
            nc.scalar.dma_start(out=bfc[:], in_=bfc_d[:])

            # PE warm-up: the HAM clock gate keeps the PE at 1.2 GHz until
            # it has seen ~3.4us of sustained matmul activity.  Run dummy
            # matmuls on a zeroed scratch tile while the first x DMA is in
            # flight so the real matmuls start at 2.4 GHz.
            scratch = const.tile([128, TN], f16, name="scratch")
            nc.gpsimd.memset(scratch[:], 0)
            for _ in range(6):
                pwarm = c1_pool.tile([112, TN], f32, tag="p1", name="p1")
                nc.tensor.matmul(pwarm[:], scratch[:, 0:112], scratch[:],
                                 start=True, stop=True)

            w1 = wsb[0:WIN, 0:112]
            w2r0 = wsb[0:112, 112:224]
            w2r1 = wsb[0:112, 224:336]
            w2r2 = wsb[0:112, 336:448]
            wfc = wsb[0:112, 448:455]
            b1 = csb[:, 0:1]
            b2 = csb[:, 1:2]

            # single-partition staging laid out in DRAM byte order
            y_sb = y_pool.tile([1, nt_tiles * TN], f32, name="y_sb")

            # round-robin the PSUM->SBUF evictions across ACT and DVE
            evict_i = [0]

            def evict_relu(dst, src, bias):
                evict_i[0] += 1
                if evict_i[0] % 2:
                    nc.vector.tensor_scalar(dst, src, bias, 0.0,
                                            OP.add, OP.max)
                else:
                    nc.scalar.activation(dst, src, AF.Relu, bias=bias)

            # alternate x DMAs across the sync and gpsimd queues so the
            # transfers overlap
            dmaq = [nc.sync, nc.gpsimd]
            dma_i = [0]

            def x_dma(dst, t, w0, nw, q=None):
                if q is None:
                    q = dmaq[dma_i[0] % 2]
                    dma_i[0] += 1
                q.dma_start(out=dst[:],
                            in_=x_d[bass.ds(t * WIN, WIN),
                                    bass.ds(w0 * TN, nw * TN)])

            def fc_mm(fcp, r, z2t):
                nc.tensor.matmul(fcp[:], wfc[:, r:r + 1], z2t[r][:],
                                 start=(r == 0), stop=(r == 6))

            def epilogue(t, fcp):
                ys = y_sb[0:1, bass.ds(t * TN, TN)]
                nc.scalar.activation(ys, fcp[:], AF.Sigmoid, bias=bfc[:, 0:1])
                # p = (1-k) + k*sigmoid(t); clip is <=1e-6 rel, dropped
                nc.vector.tensor_scalar(ys, ys, KDEC, 1.0 - KDEC,
                                        OP.mult, OP.add)

            # fc + epilogue of tile t, emitted inside tile t+1's conv1
            # stream so the PE never waits on the z2 evictions
            def flush_fc(t, z2t):
                fcp = fc_pool.tile([1, TN], f32, tag="fc", name="fc")
                for r in range(7):
                    fc_mm(fcp, r, z2t)
                epilogue(t, fcp)
                # stream the finished y values out in 4-tile chunks
                if t % 4 == 3:
                    nc.sync.dma_start(
                        out=y_d[bass.ds((t - 3) * TN, 4 * TN)],
                        in_=y_sb[0:1, bass.ds((t - 3) * TN, 4 * TN)])

            pend = None  # (tile index, z2 dict) awaiting fc flush

            for t in range(nt_tiles):
                # per-tile x: tile 0 arrives in 5 small pieces so the
                # first matmul starts as early as possible; later tiles
                # in halves (prefetched well ahead of use)
                segs = []
                if t == 0:
                    # tile 0 arrives in 5 pieces spread over 4 DMA queues
                    # so the transfers run in parallel and the PE is fed
                    # as soon as the warm-up dummies finish
                    for (tag, w0, nw), q in zip(
                            (("h0", 0, 2), ("h1", 2, 2), ("h2", 4, 3),
                             ("h3", 7, 3), ("h4", 10, 4)),
                            (nc.sync, nc.gpsimd, nc.sync, nc.gpsimd,
                             nc.scalar)):
                        xt = head_pool.tile([WIN, nw * TN], f16, tag=tag,
                                            name=tag)
                        x_dma(xt, t, w0, nw, q=q)
                        segs.append((xt, w0, nw))
                else:
                    for w0 in (0, 7):
                        xt = xt_pool.tile([WIN, 7 * TN], f16, tag="xt",
                                          name="xt")
                        x_dma(xt, t, w0, 7)
                        segs.append((xt, w0, 7))

                z1 = {}
                z2 = {}

                def c1(oi):
                    for src, w0, nw in segs:
                        if w0 <= oi < w0 + nw:
                            break
                    p1 = c1_pool.tile([112, TN], f32, tag="p1", name="p1")
                    nc.tensor.matmul(p1[:], w1,
                                     src[:, bass.ds((oi - w0) * TN, TN)],
                                     start=True, stop=True)
                    z1[oi] = z1_pool.tile([112, TN], f16, tag="z1", name="z1")
                    evict_relu(z1[oi][:], p1[:], b1)

                def chunk(rows):
                    p2 = {}
                    for r in rows:
                        p2[r] = c2_pool.tile([112, TN], f32, tag="p2",
                                             name="p2")
                        nc.tensor.matmul(p2[r][:], w2r1, z1[2 * r][:],
                                         start=True, stop=False)
                    for r in rows:
                        if r > 0:
                            nc.tensor.matmul(p2[r][:], w2r0,
                                             z1[2 * r - 1][:],
                                             start=False, stop=False)
                    for r in rows:
                        nc.tensor.matmul(p2[r][:], w2r2, z1[2 * r + 1][:],
                                         start=False, stop=True)
                    for r in rows:
                        z2[r] = z2_pool.tile([112, TN], f16, tag="z2",
                                             name="z2")
                        evict_relu(z2[r][:], p2[r][:], b2)

                last = t == nt_tiles - 1

                for oi in range(4):
                    c1(oi)
                if pend is not None:
                    flush_fc(*pend)
                    pend = None
                for oi in range(4, 7):
                    c1(oi)
                chunk([0, 1])
                for oi in range(7, 10):
                    c1(oi)
                chunk([2, 3])
                if last:
                    fcp = fc_pool.tile([1, TN], f32, tag="fc", name="fc")
                    fc_mm(fcp, 0, z2)
                    fc_mm(fcp, 1, z2)
                for oi in range(10, 14):
                    c1(oi)
                chunk([4, 5])
                if last:
                    fc_mm(fcp, 2, z2)
                    fc_mm(fcp, 3, z2)
                chunk([6])
                if last:
                    for r in (4, 5, 6):
                        fc_mm(fcp, r, z2)
                    epilogue(t, fcp)
                    nc.sync.dma_start(
                        out=y_d[bass.ds(12 * TN, 4 * TN)],
                        in_=y_sb[0:1, bass.ds(12 * TN, 4 * TN)])
                else:
                    pend = (t, z2)

    nc.finalize()
    return nc


_NC_CACHE = {}


def _get_nc(nt_tiles):
    if nt_tiles not in _NC_CACHE:
        _NC_CACHE[nt_tiles] = _build_nc(nt_tiles)
    return _NC_CACHE[nt_tiles]


def _install_trace_hook():
    """Register the axon NTFF profiling hook (test-time only)."""
    import contextlib
    import ctypes
    import sys
    import types

    if "antenv.axon_hooks" in sys.modules:
        return
    try:
        lib = ctypes.CDLL("/opt/axon/libaxon_pjrt.so")
        if not hasattr(lib, "axon_start_nrt_profile"):
            return
        lib.axon_start_nrt_profile.argtypes = [
            ctypes.POINTER(ctypes.c_int64), ctypes.c_size_t]
        lib.axon_start_nrt_profile.restype = ctypes.c_int64
        lib.axon_stop_nrt_profile.argtypes = [ctypes.c_char_p]
        lib.axon_stop_nrt_profile.restype = ctypes.c_int64

        @contextlib.contextmanager
        def _hook(output_dir, device_ids):
            import jax
            jax.devices()
            if device_ids:
                ids = (ctypes.c_int64 * len(device_ids))(*device_ids)
                rc = lib.axon_start_nrt_profile(ids, len(device_ids))
            else:
                rc = lib.axon_start_nrt_profile(None, 0)
            if rc != 0:
                raise RuntimeError(f"axon_start_nrt_profile rc={rc}")
            try:
                yield
            finally:
                rc = lib.axon_stop_nrt_profile(output_dir.encode())
                if rc not in (0, 3):
                    raise RuntimeError(f"axon_stop_nrt_profile rc={rc}")

        mod = types.ModuleType("antenv.axon_hooks")
        mod.get_axon_ntff_profile_hook = lambda: _hook
        mod.set_axon_ntff_profile_hook = lambda h: None
        sys.modules["antenv.axon_hooks"] = mod
        import concourse.bass_utils as bu
        bu.upload_artifacts = lambda tmpdir: tmpdir
    except Exception:
        pass


def kernel(x, W1, b1, W2, b2, Wfc, bfc):
    global LAST_EXEC_NS, LAST_PROFILE_JSON
    from concourse.bass_utils import run_bass_kernel_spmd

    x16 = np.asarray(x, np.float32).reshape(B, 784).astype(np.float16)
    x16pad = np.zeros((B, 840), np.float16)
    x16pad[:, 28:812] = x16
    xw_cores = _window_x(x16pad)

    wpack, cpack = _build_weight_mats(W1, b1, W2, b2, Wfc)
    bfc_a = np.asarray(bfc, np.float32).reshape(1, 1)

    nc = _get_nc(NT)
    shared = {"wpack": wpack, "cpack": cpack, "bfc": bfc_a}
    in_maps = [{"xw": xw_cores[i], **shared} for i in range(NCORES)]
    core_ids = list(range(NCORES))
    res = run_bass_kernel_spmd(nc, in_maps, core_ids)
    y = np.concatenate([res.results[i]["y"] for i in range(NCORES)])

    if TRACE:
        _install_trace_hook()
        try:
            tres = run_bass_kernel_spmd(nc, in_maps, core_ids, trace=True)
            LAST_EXEC_NS = tres.exec_time_ns
            LAST_PROFILE_JSON = tres.profile_json
        except Exception as e:  # profiling must never break the result path
            print("trace failed:", e)

    return y.astype(np.float32)


# revision 12
# speedup vs baseline: 1.0433x; 1.0433x over previous
"""Trainium2 Bass kernel for the DiffsolClassifier model.

Network (per image, NCHW fp32):
    z1 = relu(conv2d(x, W1, b1, k=3, s=2, p=1))   # [8,14,14]
    z2 = relu(conv2d(z1, W2, b2, k=3, s=2, p=1))  # [16,7,7]
    t  = flatten(z2) @ Wfc.T + bfc                # [1]
    p  = clip(1 - exp(-(softplus(t) + 1e-3)), 1e-6, 1-1e-6)
       = (1-k) + k*sigmoid(t),  k = exp(-1e-3)    (exact identity;
         the clip changes the result by <= 1e-6 rel so it is dropped)

Sharding: pure data parallel, batch 65536 split 8192/core across 8 cores.

Per-core mapping (16 outer tiles x 512 images), fp16 data / fp32 PSUM:
  - The HOST pre-lays x into conv1-window order: for each outer tile,
    xw[p, oi, n] = x[img n, pixel 56*oi - 28 + p] (zero when out of
    range), p in 0..83 covering the 3 input rows each conv1 output row
    reads.  On device this is a plain contiguous DMA (14336B per
    partition) - no DMA transpose and no per-window weight variants.
  - conv1: 14 matmuls per tile, ALL sharing one stationary weight
    matrix w1 [84, 112] (112 = 8ch x 14 cols, banded taps), so the PE
    never reloads weights within the phase.  Bias+relu fused into the
    PSUM eviction (alternating ACT/DVE), output z1 rows [112, 512] fp16.
  - conv2: output rows in chunks of 2; per chunk the 3 tap matrices
    w2r1/w2r0/w2r2 [112,112] are each used for both rows back-to-back
    (grouped to reuse the loaded weights), accumulating in fp32 PSUM.
    Chunks are interleaved between conv1 matmuls so the eviction
    engines (slower per-tile than the PE) never stall PSUM recycling.
  - FC: per z2 row a [112,1] matmul accumulated into PSUM [1,512]; the
    7 matmuls + sigmoid epilogue of tile t are emitted inside tile
    t+1's conv1 stream so the z2-eviction -> fc dependency is hidden.
  - All fp16 weights ship in ONE packed [128, 455] DMA and the fp32
    biases in one [112, 3] DMA, issued on the scalar queue so they
    overlap the first x-tile DMA on the sync queue.
"""

import numpy as np

B = 65536
NCORES = 8
BS = B // NCORES  # 8192 images per core
TN = 512          # images per outer tile
NT = BS // TN     # 16 outer tiles

KDEC = float(np.exp(np.float32(-0.001)))

# conv1 window geometry: window oi covers pixels 56*oi-28 .. 56*oi+55
WIN = 84   # window rows (3 image rows x 28 cols)
NWIN = 14  # conv1 output rows

# set by test.py for profiling; harness leaves these alone
TRACE = False
LAST_EXEC_NS = None
LAST_PROFILE_JSON = None


def _build_weight_mats(W1, b1, W2, b2, Wfc):
    """Host-side restructuring of the tiny conv/fc weights into the
    banded matrices the PE matmuls consume."""
    W1 = np.asarray(W1, np.float32).reshape(8, 1, 3, 3)
    W2 = np.asarray(W2, np.float32).reshape(16, 8, 3, 3)
    Wfc = np.asarray(Wfc, np.float32).reshape(1, 784)

    # w1row[(di,j), (co,oj)] over a 3-row x 28-col input window
    w1row = np.zeros((WIN, 112), np.float32)
    for co in range(8):
        for oj in range(14):
            m = co * 14 + oj
            for di in range(3):
                for dj in range(3):
                    j = 2 * oj - 1 + dj
                    if 0 <= j < 28:
                        w1row[di * 28 + j, m] = W1[co, 0, di, dj]

    # conv2 tap matrices: W2r[di][(ci,j), (co2,oj2)]
    W2r = np.zeros((3, 112, 112), np.float32)
    for di in range(3):
        for co in range(16):
            for oj in range(7):
                m = co * 7 + oj
                for ci in range(8):
                    for dj in range(3):
                        j = 2 * oj - 1 + dj
                        if 0 <= j < 14:
                            W2r[di, ci * 14 + j, m] = W2[co, ci, di, dj]

    # fc columns per z2 row: wfc[(co2,oj2), i2]
    wfc = np.zeros((112, 7), np.float32)
    for co in range(16):
        for i2 in range(7):
            for oj in range(7):
                wfc[co * 7 + oj, i2] = Wfc[0, co * 49 + i2 * 7 + oj]

    # one packed fp16 weight image [128, 455]:
    #   cols   0:112 w1row (rows 0:84)
    #   cols 112:224 w2r0, 224:336 w2r1, 336:448 w2r2 (rows 0:112)
    #   cols 448:455 wfc (rows 0:112)
    wpack = np.zeros((128, 455), np.float16)
    wpack[0:WIN, 0:112] = w1row
    wpack[0:112, 112:224] = W2r[0]
    wpack[0:112, 224:336] = W2r[1]
    wpack[0:112, 336:448] = W2r[2]
    wpack[0:112, 448:455] = wfc

    # packed fp32 per-partition consts [112, 3]: b1col | b2col | bfc@[0,2]
    cpack = np.zeros((112, 3), np.float32)
    cpack[:, 0] = np.repeat(np.asarray(b1, np.float32), 14)
    cpack[:, 1] = np.repeat(np.asarray(b2, np.float32), 7)
    return wpack, cpack


def _window_x(x16pad):
    """[B, 840] padded fp16 -> per-core tile-major window layout
    [NCORES][NT*84, 14*512] where row (t*84+p), col (oi*512+n) holds
    x[core*8192 + t*512 + n, pixel 56*oi - 28 + p]."""
    from numpy.lib.stride_tricks import as_strided
    v = as_strided(x16pad, shape=(B, NWIN, WIN),
                   strides=(840 * 2, 56 * 2, 2))
    full = v.reshape(NCORES, NT, TN, NWIN, WIN).transpose(0, 1, 4, 3, 2)
    out = []
    for c in range(NCORES):
        out.append(np.ascontiguousarray(full[c]).reshape(NT * WIN, NWIN * TN))
    return out


def _build_nc(nt_tiles):
    import concourse.bacc as bacc
    import concourse.bass as bass
    import concourse.mybir as mybir
    import concourse.tile as tile

    f32 = mybir.dt.float32
    f16 = mybir.dt.float16
    AF = mybir.ActivationFunctionType
    OP = mybir.AluOpType
    bs = nt_tiles * TN

    nc = bacc.Bacc(None)
    x_d = nc.declare_dram_parameter("xw", [nt_tiles * WIN, NWIN * TN], f16,
                                    isOutput=False)
    w_d = nc.declare_dram_parameter("wpack", [128, 455], f16, isOutput=False)
    c_d = nc.declare_dram_parameter("cpack", [112, 3], f32, isOutput=False)
    bfc_d = nc.declare_dram_parameter("bfc", [1, 1], f32, isOutput=False)
    y_d = nc.declare_dram_parameter("y", [bs], f32, isOutput=True)

    with tile.TileContext(nc) as tc:
        with (
            tc.tile_pool(name="const", bufs=1) as const,
            tc.tile_pool(name="head_pool", bufs=1) as head_pool,
            tc.tile_pool(name="xt_pool", bufs=6) as xt_pool,
            tc.tile_pool(name="z1_pool", bufs=16) as z1_pool,
            tc.tile_pool(name="z2_pool", bufs=16) as z2_pool,
            tc.tile_pool(name="y_pool", bufs=1) as y_pool,
            tc.tile_pool(name="c1_psum", bufs=4, space="PSUM") as c1_pool,
            tc.tile_pool(name="c2_psum", bufs=3, space="PSUM") as c2_pool,
            tc.tile_pool(name="fc_psum", bufs=1, space="PSUM") as fc_pool,
        ):
            wsb = const.tile([128, 455], f16, name="wsb")
            csb = const.tile([112, 3], f32, name="csb")
            bfc = const.tile([1, 1], f32, name="bfc")
            # weights first on the two DMA queues (sync + gpsimd); the
            # scalar/vector queues never issue DMAs - they run evictions
            # and would delay a DMA behind semaphore waits
            nc.sync.dma_start(out=wsb[:], in_=w_d[:])
            nc.gpsimd.dma_start(out=csb[:], in_=c_d[:])
            nc.gpsimd.dma_start(out=bfc[:], in_=bfc_d[:])

            # PE warm-up: the HAM clock gate keeps the PE at 1.2 GHz until
            # it has seen ~3.4us of sustained matmul activity.  Run dummy
            # matmuls on a zeroed scratch tile while the first x DMA is in
            # flight so the real matmuls start at 2.4 GHz.
            scratch = const.tile([128, TN], f16, name="scratch")
            nc.gpsimd.memset(scratch[:], 0)
            for _ in range(6):
                pwarm = c1_pool.tile([112, TN], f32, tag="p1", name="p1")
                nc.tensor.matmul(pwarm[:], scratch[:, 0:112], scratch[:],
                                 start=True, stop=True)

            w1 = wsb[0:WIN, 0:112]
            w2r0 = wsb[0:112, 112:224]
            w2r1 = wsb[0:112, 224:336]
            w2r2 = wsb[0:112, 336:448]
            wfc = wsb[0:112, 448:455]
            b1 = csb[:, 0:1]
            b2 = csb[:, 1:2]

            # single-partition staging laid out in DRAM byte order
            y_sb = y_pool.tile([1, nt_tiles * TN], f32, name="y_sb")

            # round-robin the PSUM->SBUF evictions across ACT and DVE
            evict_i = [0]

            def evict_relu(dst, src, bias):
                evict_i[0] += 1
                if evict_i[0] % 2:
                    nc.vector.tensor_scalar(dst, src, bias, 0.0,
                                            OP.add, OP.max)
                else:
                    nc.scalar.activation(dst, src, AF.Relu, bias=bias)

            # alternate x DMAs across the sync and gpsimd queues so the
            # transfers overlap
            dmaq = [nc.sync, nc.gpsimd]
            dma_i = [0]

            def x_dma(dst, t, w0, nw, q=None):
                if q is None:
                    q = dmaq[dma_i[0] % 2]
                    dma_i[0] += 1
                q.dma_start(out=dst[:],
                            in_=x_d[bass.ds(t * WIN, WIN),
                                    bass.ds(w0 * TN, nw * TN)])

            def fc_mm(fcp, r, z2t):
                nc.tensor.matmul(fcp[:], wfc[:, r:r + 1], z2t[r][:],
                                 start=(r == 0), stop=(r == 6))

            def epilogue(t, fcp):
                ys = y_sb[0:1, bass.ds(t * TN, TN)]
                nc.scalar.activation(ys, fcp[:], AF.Sigmoid, bias=bfc[:, 0:1])
                # p = (1-k) + k*sigmoid(t); clip is <=1e-6 rel, dropped
                nc.vector.tensor_scalar(ys, ys, KDEC, 1.0 - KDEC,
                                        OP.mult, OP.add)

            # fc + epilogue of tile t, emitted inside tile t+1's conv1
            # stream so the PE never waits on the z2 evictions
            def flush_fc(t, z2t):
                fcp = fc_pool.tile([1, TN], f32, tag="fc", name="fc")
                for r in range(7):
                    fc_mm(fcp, r, z2t)
                epilogue(t, fcp)
                # stream the finished y values out in 4-tile chunks
                if t % 4 == 3:
                    nc.sync.dma_start(
                        out=y_d[bass.ds((t - 3) * TN, 4 * TN)],
                        in_=y_sb[0:1, bass.ds((t - 3) * TN, 4 * TN)])

            pend = None  # (tile index, z2 dict) awaiting fc flush

            for t in range(nt_tiles):
                # per-tile x: tile 0 arrives in 5 small pieces so the
                # first matmul starts as early as possible; later tiles
                # in halves (prefetched well ahead of use)
                segs = []
                if t == 0:
                    # tile 0 arrives in 7 two-window pieces alternating
                    # between the sync and gpsimd queues so the transfers
                    # run in parallel and the PE is fed as soon as the
                    # warm-up dummies finish
                    for i in range(7):
                        xt = head_pool.tile([WIN, 2 * TN], f16,
                                            tag=f"h{i}", name=f"h{i}")
                        x_dma(xt, t, 2 * i, 2)
                        segs.append((xt, 2 * i, 2))
                else:
                    for w0 in (0, 7):
                        xt = xt_pool.tile([WIN, 7 * TN], f16, tag="xt",
                                          name="xt")
                        x_dma(xt, t, w0, 7)
                        segs.append((xt, w0, 7))

                z1 = {}
                z2 = {}

                def c1(oi):
                    for src, w0, nw in segs:
                        if w0 <= oi < w0 + nw:
                            break
                    p1 = c1_pool.tile([112, TN], f32, tag="p1", name="p1")
                    nc.tensor.matmul(p1[:], w1,
                                     src[:, bass.ds((oi - w0) * TN, TN)],
                                     start=True, stop=True)
                    z1[oi] = z1_pool.tile([112, TN], f16, tag="z1", name="z1")
                    evict_relu(z1[oi][:], p1[:], b1)

                def chunk(rows):
                    p2 = {}
                    for r in rows:
                        p2[r] = c2_pool.tile([112, TN], f32, tag="p2",
                                             name="p2")
                        nc.tensor.matmul(p2[r][:], w2r1, z1[2 * r][:],
                                         start=True, stop=False)
                    for r in rows:
                        if r > 0:
                            nc.tensor.matmul(p2[r][:], w2r0,
                                             z1[2 * r - 1][:],
                                             start=False, stop=False)
                    for r in rows:
                        nc.tensor.matmul(p2[r][:], w2r2, z1[2 * r + 1][:],
                                         start=False, stop=True)
                    for r in rows:
                        z2[r] = z2_pool.tile([112, TN], f16, tag="z2",
                                             name="z2")
                        evict_relu(z2[r][:], p2[r][:], b2)

                last = t == nt_tiles - 1

                for oi in range(4):
                    c1(oi)
                if pend is not None:
                    flush_fc(*pend)
                    pend = None
                for oi in range(4, 7):
                    c1(oi)
                chunk([0, 1])
                for oi in range(7, 10):
                    c1(oi)
                chunk([2, 3])
                if last:
                    fcp = fc_pool.tile([1, TN], f32, tag="fc", name="fc")
                    fc_mm(fcp, 0, z2)
                    fc_mm(fcp, 1, z2)
                for oi in range(10, 14):
                    c1(oi)
                chunk([4, 5])
                if last:
                    fc_mm(fcp, 2, z2)
                    fc_mm(fcp, 3, z2)
                chunk([6])
                if last:
                    for r in (4, 5, 6):
                        fc_mm(fcp, r, z2)
                    epilogue(t, fcp)
                    nc.sync.dma_start(
                        out=y_d[bass.ds(12 * TN, 4 * TN)],
                        in_=y_sb[0:1, bass.ds(12 * TN, 4 * TN)])
                else:
                    pend = (t, z2)

    nc.finalize()
    return nc


_NC_CACHE = {}


def _get_nc(nt_tiles):
    if nt_tiles not in _NC_CACHE:
        _NC_CACHE[nt_tiles] = _build_nc(nt_tiles)
    return _NC_CACHE[nt_tiles]


def _install_trace_hook():
    """Register the axon NTFF profiling hook (test-time only)."""
    import contextlib
    import ctypes
    import sys
    import types

    if "antenv.axon_hooks" in sys.modules:
        return
    try:
        lib = ctypes.CDLL("/opt/axon/libaxon_pjrt.so")
        if not hasattr(lib, "axon_start_nrt_profile"):
            return
        lib.axon_start_nrt_profile.argtypes = [
            ctypes.POINTER(ctypes.c_int64), ctypes.c_size_t]
        lib.axon_start_nrt_profile.restype = ctypes.c_int64
        lib.axon_stop_nrt_profile.argtypes = [ctypes.c_char_p]
        lib.axon_stop_nrt_profile.restype = ctypes.c_int64

        @contextlib.contextmanager
        def _hook(output_dir, device_ids):
            import jax
            jax.devices()
            if device_ids:
                ids = (ctypes.c_int64 * len(device_ids))(*device_ids)
                rc = lib.axon_start_nrt_profile(ids, len(device_ids))
            else:
                rc = lib.axon_start_nrt_profile(None, 0)
            if rc != 0:
                raise RuntimeError(f"axon_start_nrt_profile rc={rc}")
            try:
                yield
            finally:
                rc = lib.axon_stop_nrt_profile(output_dir.encode())
                if rc not in (0, 3):
                    raise RuntimeError(f"axon_stop_nrt_profile rc={rc}")

        mod = types.ModuleType("antenv.axon_hooks")
        mod.get_axon_ntff_profile_hook = lambda: _hook
        mod.set_axon_ntff_profile_hook = lambda h: None
        sys.modules["antenv.axon_hooks"] = mod
        import concourse.bass_utils as bu
        bu.upload_artifacts = lambda tmpdir: tmpdir
    except Exception:
        pass


def kernel(x, W1, b1, W2, b2, Wfc, bfc):
    global LAST_EXEC_NS, LAST_PROFILE_JSON
    from concourse.bass_utils import run_bass_kernel_spmd

    x16 = np.asarray(x, np.float32).reshape(B, 784).astype(np.float16)
    x16pad = np.zeros((B, 840), np.float16)
    x16pad[:, 28:812] = x16
    xw_cores = _window_x(x16pad)

    wpack, cpack = _build_weight_mats(W1, b1, W2, b2, Wfc)
    bfc_a = np.asarray(bfc, np.float32).reshape(1, 1)

    nc = _get_nc(NT)
    shared = {"wpack": wpack, "cpack": cpack, "bfc": bfc_a}
    in_maps = [{"xw": xw_cores[i], **shared} for i in range(NCORES)]
    core_ids = list(range(NCORES))
    res = run_bass_kernel_spmd(nc, in_maps, core_ids)
    y = np.concatenate([res.results[i]["y"] for i in range(NCORES)])

    if TRACE:
        _install_trace_hook()
        try:
            tres = run_bass_kernel_spmd(nc, in_maps, core_ids, trace=True)
            LAST_EXEC_NS = tres.exec_time_ns
            LAST_PROFILE_JSON = tres.profile_json
        except Exception as e:  # profiling must never break the result path
            print("trace failed:", e)

    return y.astype(np.float32)
